# revision 10
# baseline (speedup 1.0000x reference)
"""IVF-style clustered cosine top-k retrieval on 8 Trainium2 NeuronCores.

Problem: querys (64,768); data (1000,256,768); centers (1000,768);
clusted_idx (1000,256); k=16; num_search=10.
reference: pick 10 closest clusters per query by centroid cosine, then exact
cosine top-16 over the 2560 docs in those clusters; return (ret_idx, ret_emb).

Sharding: queries are sharded 8-per-core (each core is self-sufficient: it
selects clusters for its own queries, gathers those clusters' docs via
indirect DMA, scores them and produces its queries' final top-16). The host
only slices inputs, concatenates per-core results, and translates candidate
positions into global doc ids / embeddings (pure indexing, no arithmetic).

Device pipeline per core:
  A. cluster selection: PE matmuls q @ centersT and ones @ centersT^2
     (center norms), DVE/ACT for sqrt+reciprocal+scale, then hardware
     top-8 ops (max/max_index/match_replace) for the top-10 clusters.
  B. for each (query, selected cluster): gather the cluster's 256x768 block
     with per-partition indirect DMA (offsets = cluster_id*256 + partition),
     dot products via fused tensor_tensor_reduce (multiply+accumulate in one
     DVE pass), squared norms via ScalarE activation(Square, accum_out=...).
  C. per query: scores = dots * rsqrt(norms); 32x32 block transposes to get
     candidates on the free axis; two-level max8 tournament for the top-16.
"""

import os
import sys
import types

import numpy as np

for _p in ("/opt/trn_rl_repo", "/opt/pypackages"):
    if _p not in sys.path and os.path.isdir(_p):
        sys.path.append(_p)

import concourse.bass as bass
import concourse.bacc as bacc
import concourse.mybir as mybir
import concourse.tile as tile
from concourse import bass_utils

B, D = 64, 768
K, NPC = 1000, 256
TOPK = 16
NSEARCH = 10
NCORES = 8
QPC = B // NCORES          # queries per core
KC = D // 128              # contraction chunks (6)
NSLOT = 2 * NSEARCH        # 128-doc half-cluster slots per query (20)
NEG = -1.0e30

LAST_EXEC_NS = None
_CACHED = {}


def _install_profile_hook():
    """The agent image's antenv lacks axon_hooks; fabricate it so
    run_bass_kernel_spmd(trace=True) can capture an NTFF profile."""
    try:
        from antenv.axon_hooks import get_axon_ntff_profile_hook  # noqa: F401
        return
    except ImportError:
        pass
    try:
        from trn_agent_boot.trn_boot import _ntff_profile_via_ctypes
        hook = _ntff_profile_via_ctypes("/opt/axon/libaxon_pjrt.so")
    except Exception:
        hook = None
    mod = types.ModuleType("antenv.axon_hooks")
    mod._hook = hook
    mod.get_axon_ntff_profile_hook = lambda: mod._hook
    mod.set_axon_ntff_profile_hook = lambda h: setattr(mod, "_hook", h)
    sys.modules["antenv.axon_hooks"] = mod
    try:
        import antenv
        antenv.axon_hooks = mod
    except ImportError:
        pass


def build_program():
    nc = bacc.Bacc(
        "TRN2",
        target_bir_lowering=False,
        debug=False,
        enable_asserts=True,
        num_devices=NCORES,
    )
    f32 = mybir.dt.float32
    u32 = mybir.dt.uint32

    q_nat = nc.dram_tensor("q_nat", [QPC, D], f32, kind="ExternalInput").ap()
    qT = nc.dram_tensor("qT", [D, QPC], f32, kind="ExternalInput").ap()
    cT = nc.dram_tensor("cT", [D, K], f32, kind="ExternalInput").ap()
    data = nc.dram_tensor("data", [K * NPC, D], f32, kind="ExternalInput").ap()

    out_cidx = nc.dram_tensor("out_cidx", [QPC, 16], u32, kind="ExternalOutput").ap()
    out_val = nc.dram_tensor("out_val", [QPC, 16], f32, kind="ExternalOutput").ap()
    out_pos = nc.dram_tensor("out_pos", [QPC, 16], u32, kind="ExternalOutput").ap()
    out_l1i = nc.dram_tensor("out_l1i", [QPC, NSLOT * 16], u32, kind="ExternalOutput").ap()

    with tile.TileContext(nc) as tc:
        with (
            tc.tile_pool(name="persist", bufs=1) as pp,
            tc.tile_pool(name="selpsum", bufs=1, space="PSUM") as selp,
            tc.tile_pool(name="selbuf", bufs=1) as selb,
            tc.tile_pool(name="cl", bufs=6) as clp,
            tc.tile_pool(name="qacc", bufs=2) as qap,
            tc.tile_pool(name="topk", bufs=2) as tkp,
        ):
            # ---------- stage A: cluster selection ----------
            q_flat = pp.tile([1, QPC * D], f32)
            nc.sync.dma_start(q_flat[0:1, :], q_nat.rearrange("q d -> (q d)"))
            qT_sb = pp.tile([128, KC, QPC], f32)
            nc.sync.dma_start(qT_sb[:], qT.rearrange("(c p) q -> p c q", p=128))
            cT_sb = selb.tile([128, KC, K], f32)
            nc.sync.dma_start(cT_sb[:], cT.rearrange("(c p) n -> p c n", p=128))

            ones_sb = pp.tile([128, 1], f32)
            nc.vector.memset(ones_sb[:], 1.0)
            iota_p = pp.tile([128, 1], u32)
            nc.gpsimd.iota(iota_p[:], pattern=[[0, 1]], base=0, channel_multiplier=1)

            sq_cT = selb.tile([128, KC, K], f32)
            nc.scalar.square(sq_cT[:], cT_sb[:])

            ps_c = [selp.tile([QPC, 512], f32, name=f"ps_c{i}", tag=f"ps_c{i}") for i in range(2)]
            ps_n = [selp.tile([1, 512], f32, name=f"ps_n{i}", tag=f"ps_n{i}") for i in range(2)]
            for h in range(2):
                cols = slice(h * 500, (h + 1) * 500)
                for c in range(KC):
                    nc.tensor.matmul(
                        ps_c[h][:, 0:500],
                        lhsT=qT_sb[:, c, :],
                        rhs=cT_sb[:, c, cols],
                        start=(c == 0),
                        stop=(c == KC - 1),
                    )
                for c in range(KC):
                    nc.tensor.matmul(
                        ps_n[h][:, 0:500],
                        lhsT=ones_sb[:],
                        rhs=sq_cT[:, c, cols],
                        start=(c == 0),
                        stop=(c == KC - 1),
                    )

            # scores = dot / ||center|| ; per-query scale ||q|| is rank-invariant
            rn_c = selb.tile([1, K], f32)
            sc_c = selb.tile([QPC, K], f32)
            for h in range(2):
                cols = slice(h * 500, (h + 1) * 500)
                nc.scalar.sqrt(rn_c[:, cols], ps_n[h][:, 0:500])
            nc.vector.reciprocal(rn_c[:], rn_c[:])
            rn_bc = selb.tile([QPC, K], f32)
            nc.gpsimd.partition_broadcast(rn_bc[:], rn_c[:])
            for h in range(2):
                cols = slice(h * 500, (h + 1) * 500)
                nc.vector.tensor_tensor(
                    out=sc_c[:, cols],
                    in0=ps_c[h][:, 0:500],
                    in1=rn_bc[:, cols],
                    op=mybir.AluOpType.mult,
                )

            cidx_sb = pp.tile([QPC, 16], u32)
            cval_sb = selb.tile([QPC, 16], f32)
            nc.vector.max(cval_sb[:, 0:8], sc_c[:])
            nc.vector.max_index(cidx_sb[:, 0:8], cval_sb[:, 0:8], sc_c[:])
            nc.vector.match_replace(sc_c[:], cval_sb[:, 0:8], sc_c[:], NEG)
            nc.vector.max(cval_sb[:, 8:16], sc_c[:])
            nc.vector.max_index(cidx_sb[:, 8:16], cval_sb[:, 8:16], sc_c[:])
            nc.sync.dma_start(out_cidx[:], cidx_sb[:])

            # per-(query,slot) doc-row offsets: cidx*256 + partition
            docid = pp.tile([128, QPC, NSEARCH], u32)
            cidx_flat = pp.tile([1, QPC * 16], u32)
            nc.sync.dma_start(cidx_flat[0:1, :], cidx_sb[:, :])
            cidx_bc = pp.tile([128, QPC, 16], u32)
            nc.gpsimd.partition_broadcast(
                cidx_bc[:].rearrange("p q r -> p (q r)"), cidx_flat[0:1, :]
            )
            for qi in range(QPC):
                for s in range(NSEARCH):
                    nc.vector.scalar_tensor_tensor(
                        out=docid[:, qi, s : s + 1],
                        in0=cidx_bc[:, qi, s : s + 1],
                        scalar=float(NPC),
                        in1=iota_p[:],
                        op0=mybir.AluOpType.mult,
                        op1=mybir.AluOpType.add,
                    )

            # ---------- stages B+C per query ----------
            fin_v = pp.tile([QPC, NSLOT * 16], f32)
            fin_i = pp.tile([QPC, NSLOT * 16], u32)
            for qi in range(QPC):
                dots = qap.tile([128, NSLOT], f32, tag="dots")
                nrm2 = qap.tile([128, NSLOT], f32, tag="nrm2")
                sqs = qap.tile([128, D], f32, tag="sqs")
                sqd = qap.tile([128, D], f32, tag="sqd")
                qb = qap.tile([128, D], f32, tag="qb")
                nc.gpsimd.partition_broadcast(
                    qb[:], q_flat[0:1, qi * D : (qi + 1) * D]
                )
                qrow = qb[:]

                for s in range(NSEARCH):
                    clt = clp.tile([128, 2, D], f32)
                    for hh in range(2):
                        nc.gpsimd.indirect_dma_start(
                            out=clt[:, hh, :],
                            out_offset=None,
                            in_=data[:],
                            in_offset=bass.IndirectOffsetOnAxis(
                                ap=docid[:, qi, s : s + 1], axis=0
                            ),
                            element_offset=hh * 128 * D,
                        )
                    for hh in range(2):
                        col = 2 * s + hh
                        nc.vector.scalar_tensor_tensor(
                            out=sqd[:],
                            in0=clt[:, hh, :],
                            scalar=1.0,
                            in1=qrow,
                            op0=mybir.AluOpType.mult,
                            op1=mybir.AluOpType.mult,
                            accum_out=dots[:, col : col + 1],
                        )
                        nc.scalar.activation(
                            out=sqs[:],
                            in_=clt[:, hh, :],
                            func=mybir.ActivationFunctionType.Square,
                            accum_out=nrm2[:, col : col + 1],
                        )

                # scores: dots * rsqrt(nrm2)
                sc = tkp.tile([128, 32], f32, tag="sc")
                rn = qap.tile([128, NSLOT], f32, tag="rn")
                nc.scalar.sqrt(rn[:], nrm2[:])
                nc.vector.reciprocal(rn[:], rn[:])
                nc.vector.memset(sc[:, NSLOT:32], NEG)
                nc.vector.tensor_tensor(
                    out=sc[:, 0:NSLOT], in0=dots[:], in1=rn[:],
                    op=mybir.AluOpType.mult,
                )

                # transpose to [32, 128]: candidates on the free axis
                scT = tkp.tile([32, 128], f32, tag="scT")
                for g in range(4):
                    nc.vector.transpose(
                        scT[0:32, g * 32 : (g + 1) * 32],
                        sc[g * 32 : (g + 1) * 32, 0:32],
                    )

                # L1: top-16 within each 128-doc slot row
                m1 = tkp.tile([32, 16], f32, tag="m1")
                i1 = tkp.tile([32, 16], u32, tag="i1")
                nc.vector.max(m1[:, 0:8], scT[:])
                nc.vector.max_index(i1[:, 0:8], m1[:, 0:8], scT[:])
                nc.vector.match_replace(scT[:], m1[:, 0:8], scT[:], NEG)
                nc.vector.max(m1[:, 8:16], scT[:])
                nc.vector.max_index(i1[:, 8:16], m1[:, 8:16], scT[:])

                # collect the 20x16 finalists into this query's row
                nc.sync.dma_start(fin_v[qi : qi + 1, :], m1[0:NSLOT, :])
                nc.sync.dma_start(fin_i[qi : qi + 1, :], i1[0:NSLOT, :])

            # ---------- L2: global top-16 of 320 finalists, all queries ----------
            nc.sync.dma_start(out_l1i[:], fin_i[:])
            v2 = pp.tile([QPC, 16], f32)
            p2 = pp.tile([QPC, 16], u32)
            nc.vector.max(v2[:, 0:8], fin_v[:])
            nc.vector.max_index(p2[:, 0:8], v2[:, 0:8], fin_v[:])
            nc.vector.match_replace(fin_v[:], v2[:, 0:8], fin_v[:], NEG)
            nc.vector.max(v2[:, 8:16], fin_v[:])
            nc.vector.max_index(p2[:, 8:16], v2[:, 8:16], fin_v[:])
            nc.sync.dma_start(out_val[:], v2[:])
            nc.sync.dma_start(out_pos[:], p2[:])

    nc.compile()
    return nc


def kernel(querys, data, centers, clusted_idx, k, num_search, **_unused):
    global LAST_EXEC_NS
    querys = np.ascontiguousarray(np.asarray(querys, dtype=np.float32))
    data_np = np.ascontiguousarray(np.asarray(data, dtype=np.float32))
    centers = np.ascontiguousarray(np.asarray(centers, dtype=np.float32))
    clusted_idx = np.asarray(clusted_idx)
    assert querys.shape == (B, D) and data_np.shape == (K, NPC, D)
    assert int(k) == TOPK and int(num_search) == NSEARCH

    _install_profile_hook()
    if "nc" not in _CACHED:
        _CACHED["nc"] = build_program()
    nc = _CACHED["nc"]

    cT = np.ascontiguousarray(centers.T)
    data_flat = data_np.reshape(K * NPC, D)
    in_maps = []
    for m in range(NCORES):
        qs = querys[m * QPC : (m + 1) * QPC]
        in_maps.append(
            {
                "q_nat": qs,
                "qT": np.ascontiguousarray(qs.T),
                "cT": cT,
                "data": data_flat,
            }
        )

    trace = bool(os.environ.get("BASS_TRACE"))
    res = bass_utils.run_bass_kernel_spmd(
        nc, in_maps, core_ids=list(range(NCORES)), trace=trace
    )
    LAST_EXEC_NS = res.exec_time_ns

    ret_idx = np.empty((B, TOPK), dtype=clusted_idx.dtype)
    ret_emb = np.empty((B, TOPK, D), dtype=np.float32)
    for m in range(NCORES):
        r = res.results[m]
        cidx = r["out_cidx"]
        pos = r["out_pos"]
        l1i = r["out_l1i"]
        for qi in range(QPC):
            b = m * QPC + qi
            p2 = pos[qi].astype(np.int64)          # position into the 320 finalists
            s = p2 // 16                           # half-cluster slot 0..19
            l = l1i[qi, p2].astype(np.int64)       # doc offset 0..127 within slot
            cs = s // 2                            # cluster slot 0..9
            n = (s % 2) * 128 + l                  # doc index within cluster
            c = cidx[qi, cs].astype(np.int64)      # global cluster id
            ret_idx[b] = clusted_idx[c, n]
            ret_emb[b] = data_np[c, n]
    return ret_idx, ret_emb
